# revision 4
# baseline (speedup 1.0000x reference)
"""Trainium2 Bass kernel for nn_ChaosClock (B=512, T=512, D=64, S=8, C=1000).

Mathematical collapse of the reference scan
-------------------------------------------
The reference runs a 512-step GRU scan over a ring buffer of 4096 slots with
teleporters at {0, 1024, 2048, 3072} and reads ONLY those 4 slots at the end.
ptr starts at 0 (a teleporter). A jump lands at tel[rnd] and then ptr
increments, so the position after step 0 is in {1, 1025, 2049, 3073}. From
there ptr only increments by 1 per step, and with only 511 steps remaining it
can never travel the >=511 positions needed to reach the next multiple of
1024. Consequences (hold for ANY input values at these shapes):
  * slot 0 is written exactly once, at step 0, with gru(x[:,0,:], h=0);
  * slots 1024/2048/3072 are never written (a jump to tel lands at tel+1);
  * no slot is ever read after being written, so h=0 at every step.
Therefore  logits = gru_h0(x[:,0,:]) @ Wh[:, :8].T + bh  exactly (verified
bit-exact against a NumPy port of the reference loop).

With h = 0 the GRU reduces to (biases folded on host):
  g   = x0 @ (W_ih @ Wp).T                              # (B, 24)
  r   = sigmoid(g_r + b_r)          b_r  = (W_ih@bp + b_ih + b_hh)[0:8]
  omz = sigmoid(-(g_z) + b_z')      b_z' = -(W_ih@bp + b_ih + b_hh)[8:16]
  n   = tanh(g_n + b_n + r*b_hn)    b_n  = (W_ih@bp + b_ih)[16:24], b_hn = b_hh[16:24]
  logits = (omz * n) @ Wh[:, :8].T + bh

Sharding: pure data parallel, batch 512 -> 64 rows per core on 8 cores.
Intermediates use the transposed layout (gate features on partitions, batch
on the free dim) so the gates matmul feeds the activations directly.

Written in raw-Bass Block style (explicit per-engine programs + standalone
wait_ge instructions): the axon/walrus codegen path supports at most ONE
semaphore wait per instruction, which rules out Tile's auto-generated
multi-wait sync_info.
"""

import numpy as np

_N_CORES = 8
_B = 512
_D = 64
_S = 8
_C = 1000
_BS = _B // _N_CORES  # 64 batch rows per core

_cache = {}


def _build_module():
    import concourse.bass as bass
    import concourse.mybir as mybir

    f32 = mybir.dt.float32
    Sigmoid = mybir.ActivationFunctionType.Sigmoid
    Tanh = mybir.ActivationFunctionType.Tanh
    mult = mybir.AluOpType.mult
    add = mybir.AluOpType.add

    nc = bass.Bass("TRN2", debug=False, num_devices=_N_CORES)

    # packed input, one DMA: cols 0:64 x0.T shard | 64:88 (W_ih@Wp).T | 88:92 bias cols
    packed = nc.declare_dram_parameter("packed", [_D, 92], f32, isOutput=False)
    whb = nc.declare_dram_parameter("whb", [_S + 1, _C], f32, isOutput=False)
    out = nc.declare_dram_parameter("out", [_BS, _C], f32, isOutput=True)

    packed_sb = nc.alloc_sbuf_tensor("packed_sb", [_D, 92], f32)
    whb_sb = nc.alloc_sbuf_tensor("whb_sb", [_S + 1, _C], f32)
    r_sb = nc.alloc_sbuf_tensor("r_sb", [_S, _BS], f32)
    omz_sb = nc.alloc_sbuf_tensor("omz_sb", [_S, _BS], f32)
    npre_sb = nc.alloc_sbuf_tensor("npre_sb", [_S, _BS], f32)
    n_sb = nc.alloc_sbuf_tensor("n_sb", [_S, _BS], f32)
    lhs_sb = nc.alloc_sbuf_tensor("lhs_sb", [_S + 1, _BS], f32)
    out_sb = nc.alloc_sbuf_tensor("out_sb", [_BS, _C], f32)

    gr_ps = nc.alloc_psum_tensor("gr_ps", [_S, _BS], f32)
    gz_ps = nc.alloc_psum_tensor("gz_ps", [_S, _BS], f32)
    gn_ps = nc.alloc_psum_tensor("gn_ps", [_S, _BS], f32)
    o0_ps = nc.alloc_psum_tensor("o0_ps", [_BS, 512], f32)
    o1_ps = nc.alloc_psum_tensor("o1_ps", [_BS, _C - 512], f32)

    x0t = packed_sb[:, 0:_D]
    wfr = packed_sb[:, _D:_D + _S]
    wfz = packed_sb[:, _D + _S:_D + 2 * _S]
    wfn = packed_sb[:, _D + 2 * _S:_D + 3 * _S]
    b_r = packed_sb[0:_S, 88:89]
    b_z = packed_sb[0:_S, 89:90]
    b_n = packed_sb[0:_S, 90:91]
    b_hn = packed_sb[0:_S, 91:92]

    with nc.cleanup_on_exit():
        sd = nc.alloc_semaphore("sd")  # input DMAs, +16 each, total 32
        sp = nc.alloc_semaphore("sp")  # PE milestones
        sa = nc.alloc_semaphore("sa")  # ACT milestones
        sv = nc.alloc_semaphore("sv")  # DVE milestones
        so = nc.alloc_semaphore("so")  # output DMAs, +16 each, total 64

        with nc.Block("chaos") as block:

            @block.sync
            def _(eng):
                eng.dma_start(packed_sb[:], packed[:]).then_inc(sd, 16)
                eng.dma_start(whb_sb[:], whb[:]).then_inc(sd, 16)
                # left half of the logits is ready after ACT copy #4
                eng.wait_ge(sa, 4)
                eng.dma_start(out[0:32, 0:512], out_sb[0:32, 0:512]).then_inc(so, 16)
                eng.dma_start(out[32:64, 0:512], out_sb[32:64, 0:512]).then_inc(so, 16)
                eng.wait_ge(sa, 5)
                eng.dma_start(out[0:32, 512:_C], out_sb[0:32, 512:_C]).then_inc(so, 16)
                eng.dma_start(out[32:64, 512:_C], out_sb[32:64, 512:_C]).then_inc(so, 16)
                eng.wait_ge(so, 64)

            @block.tensor
            def _(eng):
                eng.wait_ge(sd, 32)
                eng.matmul(gr_ps[:], wfr, x0t, start=True, stop=True).then_inc(sp)
                eng.matmul(gz_ps[:], wfz, x0t, start=True, stop=True).then_inc(sp)
                eng.matmul(gn_ps[:], wfn, x0t, start=True, stop=True).then_inc(sp)
                eng.wait_ge(sv, 3)  # lhs_sb complete (ones row + upd rows)
                eng.matmul(o0_ps[:], lhs_sb[:], whb_sb[:, 0:512],
                           start=True, stop=True).then_inc(sp)
                eng.matmul(o1_ps[:], lhs_sb[:], whb_sb[:, 512:_C],
                           start=True, stop=True).then_inc(sp)

            @block.scalar
            def _(eng):
                eng.wait_ge(sd, 32)  # bias columns
                eng.wait_ge(sp, 1)
                eng.activation(r_sb[:], gr_ps[:], Sigmoid, bias=b_r).then_inc(sa)
                eng.wait_ge(sp, 2)
                # 1 - sigmoid(t) == sigmoid(-t): fold "1-z" into scale=-1
                eng.activation(omz_sb[:], gz_ps[:], Sigmoid, bias=b_z,
                               scale=-1.0).then_inc(sa)
                eng.wait_ge(sv, 2)
                eng.activation(n_sb[:], npre_sb[:], Tanh, bias=b_n).then_inc(sa)
                eng.wait_ge(sp, 4)
                eng.copy(out_sb[:, 0:512], o0_ps[:]).then_inc(sa)
                eng.wait_ge(sp, 5)
                eng.copy(out_sb[:, 512:_C], o1_ps[:]).then_inc(sa)

            @block.vector
            def _(eng):
                # head-matmul lhsT: rows 0:8 become upd, row 8 stays 1.0 so
                # the bh row of whb adds the output bias
                eng.memset(lhs_sb[:], 1.0).then_inc(sv)
                eng.wait_ge(sd, 32)
                eng.wait_ge(sa, 1)
                eng.wait_ge(sp, 3)
                # npre = r * b_hn + g_n
                eng.scalar_tensor_tensor(npre_sb[:], r_sb[:], b_hn, gn_ps[:],
                                         mult, add).then_inc(sv)
                eng.wait_ge(sa, 3)
                eng.tensor_mul(lhs_sb[0:_S, :], omz_sb[:], n_sb[:]).then_inc(sv)

    return nc


def _get_module():
    if "nc" not in _cache:
        _cache["nc"] = _build_module()
    return _cache["nc"]


def _host_prep(x, Wp, bp, W_ih, b_ih, b_hh, Wh, bh):
    """Fold the pre-GRU linear chain into one packed weight/bias block."""
    f32 = np.float32
    x0t = np.ascontiguousarray(x[:, 0, :].T.astype(f32, copy=False))  # (D, B)
    wf = (W_ih @ Wp).T.astype(f32)                                    # (D, 24)
    gbias = (W_ih @ bp + b_ih).astype(f32)                            # (24,)
    packed_const = np.zeros((_D, 92 - _D), f32)
    packed_const[:, 0:3 * _S] = wf
    packed_const[0:_S, 24] = gbias[0:_S] + b_hh[0:_S]
    packed_const[0:_S, 25] = -(gbias[_S:2 * _S] + b_hh[_S:2 * _S])
    packed_const[0:_S, 26] = gbias[2 * _S:3 * _S]
    packed_const[0:_S, 27] = b_hh[2 * _S:3 * _S]
    whb = np.concatenate([Wh[:, :_S].T, bh[None, :]], axis=0).astype(f32)  # (9,1000)
    return x0t, packed_const, whb


def _make_in_maps(inputs):
    x = np.asarray(inputs["x"], dtype=np.float32)
    x0t, packed_const, whb = _host_prep(
        x,
        np.asarray(inputs["Wp"], dtype=np.float32),
        np.asarray(inputs["bp"], dtype=np.float32),
        np.asarray(inputs["W_ih"], dtype=np.float32),
        np.asarray(inputs["b_ih"], dtype=np.float32),
        np.asarray(inputs["b_hh"], dtype=np.float32),
        np.asarray(inputs["Wh"], dtype=np.float32),
        np.asarray(inputs["bh"], dtype=np.float32),
    )
    in_maps = []
    for c in range(_N_CORES):
        packed = np.concatenate(
            [x0t[:, c * _BS:(c + 1) * _BS], packed_const], axis=1)
        in_maps.append({"packed": np.ascontiguousarray(packed), "whb": whb})
    return in_maps


def kernel(**inputs):
    from concourse.bass_utils import run_bass_kernel_spmd

    in_maps = _make_in_maps(inputs)
    res = run_bass_kernel_spmd(_get_module(), in_maps, list(range(_N_CORES)))
    out = np.concatenate([res.results[c]["out"] for c in range(_N_CORES)], axis=0)
    return out.astype(np.float32, copy=False)


def run_traced(inputs, **trace_kwargs):
    """test.py helper: same as kernel() but returns (out, BassKernelResults)."""
    from concourse.bass_utils import run_bass_kernel_spmd

    in_maps = _make_in_maps(inputs)
    res = run_bass_kernel_spmd(_get_module(), in_maps, list(range(_N_CORES)),
                               trace=True, **trace_kwargs)
    out = np.concatenate([res.results[c]["out"] for c in range(_N_CORES)], axis=0)
    return out, res


# revision 5
# speedup vs baseline: 1.0733x; 1.0733x over previous
"""Trainium2 Bass kernel for nn_ChaosClock (B=512, T=512, D=64, S=8, C=1000).

Mathematical collapse of the reference scan
-------------------------------------------
The reference runs a 512-step GRU scan over a ring buffer of 4096 slots with
teleporters at {0, 1024, 2048, 3072} and reads ONLY those 4 slots at the end.
ptr starts at 0 (a teleporter). A jump lands at tel[rnd] and then ptr
increments, so the position after step 0 is in {1, 1025, 2049, 3073}. From
there ptr only increments by 1 per step, and with only 511 steps remaining it
can never travel the >=511 positions needed to reach the next multiple of
1024. Consequences (hold for ANY input values at these shapes):
  * slot 0 is written exactly once, at step 0, with gru(x[:,0,:], h=0);
  * slots 1024/2048/3072 are never written (a jump to tel lands at tel+1);
  * no slot is ever read after being written, so h=0 at every step.
Therefore  logits = gru_h0(x[:,0,:]) @ Wh[:, :8].T + bh  exactly (verified
bit-exact against a NumPy port of the reference loop).

With h = 0 the GRU reduces to (biases folded on host):
  g   = x0 @ (W_ih @ Wp).T                              # (B, 24)
  r   = sigmoid(g_r + b_r)          b_r  = (W_ih@bp + b_ih + b_hh)[0:8]
  omz = sigmoid(-(g_z) + b_z')      b_z' = -(W_ih@bp + b_ih + b_hh)[8:16]
  n   = tanh(g_n + b_n + r*b_hn)    b_n  = (W_ih@bp + b_ih)[16:24], b_hn = b_hh[16:24]
  logits = (omz * n) @ Wh[:, :8].T + bh

Sharding: pure data parallel, batch 512 -> 64 rows per core on 8 cores.
Intermediates use the transposed layout (gate features on partitions, batch
on the free dim) so the gates matmul feeds the activations directly.

Written in raw-Bass Block style (explicit per-engine programs + standalone
wait_ge instructions): the axon/walrus codegen path supports at most ONE
semaphore wait per instruction, which rules out Tile's auto-generated
multi-wait sync_info. Compute-engine accesses must start at partition
0/32/64/96, so the three gate groups sit at partitions 0/32/64 of one fused
matmul output.
"""

import numpy as np

_N_CORES = 8
_B = 512
_D = 64
_S = 8
_C = 1000
_BS = _B // _N_CORES  # 64 batch rows per core
_PK = 140             # packed input columns: 64 x0t | 72 wf | 4 bias

_cache = {}


def _build_module():
    import concourse.bass as bass
    import concourse.mybir as mybir

    f32 = mybir.dt.float32
    Sigmoid = mybir.ActivationFunctionType.Sigmoid
    Tanh = mybir.ActivationFunctionType.Tanh
    mult = mybir.AluOpType.mult
    add = mybir.AluOpType.add

    nc = bass.Bass("TRN2", debug=False, num_devices=_N_CORES)

    packed = nc.declare_dram_parameter("packed", [_D, _PK], f32, isOutput=False)
    whb = nc.declare_dram_parameter("whb", [_S + 1, _C], f32, isOutput=False)
    out = nc.declare_dram_parameter("out", [_BS, _C], f32, isOutput=True)

    packed_sb = nc.alloc_sbuf_tensor("packed_sb", [_D, _PK], f32)
    whb_sb = nc.alloc_sbuf_tensor("whb_sb", [_S + 1, _C], f32)
    r_sb = nc.alloc_sbuf_tensor("r_sb", [_S, _BS], f32)
    omz_sb = nc.alloc_sbuf_tensor("omz_sb", [_S, _BS], f32)
    npre_sb = nc.alloc_sbuf_tensor("npre_sb", [_S, _BS], f32)
    n_sb = nc.alloc_sbuf_tensor("n_sb", [_S, _BS], f32)
    lhs_sb = nc.alloc_sbuf_tensor("lhs_sb", [_S + 1, _BS], f32)
    out_sb = nc.alloc_sbuf_tensor("out_sb", [_BS, _C], f32)
    scr_sb = nc.alloc_sbuf_tensor("scr_sb", [1, 8], f32)

    g_ps = nc.alloc_psum_tensor("g_ps", [72, _BS], f32)
    o0_ps = nc.alloc_psum_tensor("o0_ps", [_BS, 512], f32)
    o1_ps = nc.alloc_psum_tensor("o1_ps", [_BS, _C - 512], f32)

    x0t = packed_sb[:, 0:_D]
    wf72 = packed_sb[:, _D:_D + 72]
    b_r = packed_sb[0:_S, 136:137]
    b_z = packed_sb[0:_S, 137:138]
    b_n = packed_sb[0:_S, 138:139]
    b_hn = packed_sb[0:_S, 139:140]
    # gate groups at quad-aligned partitions of the fused matmul output
    gr = g_ps[0:_S, :]
    gz = g_ps[32:32 + _S, :]
    gn = g_ps[64:64 + _S, :]

    with nc.cleanup_on_exit():
        sdp = nc.alloc_semaphore("sdp")  # packed DMA (+16)
        sdw = nc.alloc_semaphore("sdw")  # whb DMA (+16)
        sp = nc.alloc_semaphore("sp")    # PE milestones
        sa = nc.alloc_semaphore("sa")    # ACT milestones
        sv = nc.alloc_semaphore("sv")    # DVE milestones
        so = nc.alloc_semaphore("so")    # output DMAs (4 x +16)

        with nc.Block("chaos") as block:

            @block.sync
            def _(eng):
                eng.dma_start(packed_sb[:], packed[:]).then_inc(sdp, 16)
                eng.dma_start(whb_sb[:], whb[:]).then_inc(sdw, 16)
                eng.wait_ge(sa, 4)  # copy0 done: left half staged
                eng.dma_start(out[0:32, 0:512], out_sb[0:32, 0:512]).then_inc(so, 16)
                eng.dma_start(out[32:64, 0:512], out_sb[32:64, 0:512]).then_inc(so, 16)
                eng.wait_ge(so, 64)

            @block.tensor
            def _(eng):
                eng.wait_ge(sdp, 16)
                eng.matmul(g_ps[:], wf72, x0t, start=True, stop=True).then_inc(sp)
                eng.wait_ge(sv, 3)   # lhs_sb complete (ones row + upd rows)
                eng.wait_ge(sdw, 16)
                eng.matmul(o0_ps[:], lhs_sb[:], whb_sb[:, 0:512],
                           start=True, stop=True).then_inc(sp)
                eng.matmul(o1_ps[:], lhs_sb[:], whb_sb[:, 512:_C],
                           start=True, stop=True).then_inc(sp)

            @block.scalar
            def _(eng):
                # dummy op: pull the ACT function table load off the critical
                # path (overlaps the input DMAs); operands are scratch garbage
                eng.activation(scr_sb[0:1, 0:4], scr_sb[0:1, 0:4], Sigmoid,
                               bias=scr_sb[0:1, 4:5])
                eng.wait_ge(sp, 1)   # implies packed DMA done (PE waited it)
                eng.activation(r_sb[:], gr, Sigmoid, bias=b_r).then_inc(sa)
                # 1 - sigmoid(t) == sigmoid(-t): fold "1-z" into scale=-1
                eng.activation(omz_sb[:], gz, Sigmoid, bias=b_z,
                               scale=-1.0).then_inc(sa)
                eng.wait_ge(sv, 2)
                eng.activation(n_sb[:], npre_sb[:], Tanh, bias=b_n).then_inc(sa)
                eng.wait_ge(sp, 2)
                eng.copy(out_sb[:, 0:512], o0_ps[:]).then_inc(sa)

            @block.vector
            def _(eng):
                # head-matmul lhsT: rows 0:8 become upd, row 8 stays 1.0 so
                # the bh row of whb adds the output bias
                eng.memset(lhs_sb[:], 1.0).then_inc(sv)
                # sa>=1 transitively implies PE mm_g done and packed DMA done
                eng.wait_ge(sa, 1)
                eng.scalar_tensor_tensor(npre_sb[:], r_sb[:], b_hn, gn,
                                         mult, add).then_inc(sv)
                eng.wait_ge(sa, 3)
                eng.tensor_mul(lhs_sb[0:_S, :], omz_sb[:], n_sb[:]).then_inc(sv)
                eng.wait_ge(sp, 3)
                eng.tensor_copy(out_sb[:, 512:_C], o1_ps[:]).then_inc(sv)

            @block.gpsimd
            def _(eng):
                eng.wait_ge(sv, 4)  # copy1 done: right half staged
                eng.dma_start(out[0:32, 512:_C], out_sb[0:32, 512:_C]).then_inc(so, 16)
                eng.dma_start(out[32:64, 512:_C], out_sb[32:64, 512:_C]).then_inc(so, 16)

    return nc


def _get_module():
    if "nc" not in _cache:
        _cache["nc"] = _build_module()
    return _cache["nc"]


def _host_prep(x, Wp, bp, W_ih, b_ih, b_hh, Wh, bh):
    """Fold the pre-GRU linear chain into one packed weight/bias block."""
    f32 = np.float32
    x0t = np.ascontiguousarray(x[:, 0, :].T.astype(f32, copy=False))  # (D, B)
    wf = (W_ih @ Wp).T.astype(f32)                                    # (D, 24)
    gbias = (W_ih @ bp + b_ih).astype(f32)                            # (24,)
    pc = np.zeros((_D, _PK - _D), f32)
    pc[:, 0:_S] = wf[:, 0:_S]                 # r weights -> psum partitions 0:8
    pc[:, 32:32 + _S] = wf[:, _S:2 * _S]      # z weights -> partitions 32:40
    pc[:, 64:64 + _S] = wf[:, 2 * _S:3 * _S]  # n weights -> partitions 64:72
    pc[0:_S, 72] = gbias[0:_S] + b_hh[0:_S]
    pc[0:_S, 73] = -(gbias[_S:2 * _S] + b_hh[_S:2 * _S])
    pc[0:_S, 74] = gbias[2 * _S:3 * _S]
    pc[0:_S, 75] = b_hh[2 * _S:3 * _S]
    whb = np.concatenate([Wh[:, :_S].T, bh[None, :]], axis=0).astype(f32)  # (9,1000)
    return x0t, pc, whb


def _make_in_maps(inputs):
    x = np.asarray(inputs["x"], dtype=np.float32)
    x0t, pc, whb = _host_prep(
        x,
        np.asarray(inputs["Wp"], dtype=np.float32),
        np.asarray(inputs["bp"], dtype=np.float32),
        np.asarray(inputs["W_ih"], dtype=np.float32),
        np.asarray(inputs["b_ih"], dtype=np.float32),
        np.asarray(inputs["b_hh"], dtype=np.float32),
        np.asarray(inputs["Wh"], dtype=np.float32),
        np.asarray(inputs["bh"], dtype=np.float32),
    )
    in_maps = []
    for c in range(_N_CORES):
        packed = np.concatenate([x0t[:, c * _BS:(c + 1) * _BS], pc], axis=1)
        in_maps.append({"packed": np.ascontiguousarray(packed), "whb": whb})
    return in_maps


def kernel(**inputs):
    from concourse.bass_utils import run_bass_kernel_spmd

    in_maps = _make_in_maps(inputs)
    res = run_bass_kernel_spmd(_get_module(), in_maps, list(range(_N_CORES)))
    out = np.concatenate([res.results[c]["out"] for c in range(_N_CORES)], axis=0)
    return out.astype(np.float32, copy=False)


def run_traced(inputs, **trace_kwargs):
    """test.py helper: same as kernel() but returns (out, BassKernelResults)."""
    from concourse.bass_utils import run_bass_kernel_spmd

    in_maps = _make_in_maps(inputs)
    res = run_bass_kernel_spmd(_get_module(), in_maps, list(range(_N_CORES)),
                               trace=True, **trace_kwargs)
    out = np.concatenate([res.results[c]["out"] for c in range(_N_CORES)], axis=0)
    return out, res
